# revision 89
# baseline (speedup 1.0000x reference)
"""CPQuadRankLayer Trainium2 kernel, bf16 wire format.

Math (per node n, batch b):
  P[b,c,r]  = sum_i x[b,n,c,i] * factors[c,n,r,i]
  p         = P / sqrt(mean_r P^2 + eps)
  merged    = p0*p1*p2*p3 * gain[n]
  out[b,o]  = sum_r merged[b,r] * factor_out[n,r,o] + mean_c x[b,n,c,o]

Distribution: nodes sharded 1024 -> 8 cores x 128 nodes (node
independent, no collectives). All wire tensors are cast to bf16 on the
host, halving HBM traffic (the dominant cost; target_regime=memory).
x is pre-scaled by 0.25 on the host: p is invariant to scaling x
(the rms rescales identically), and the residual mean_c x becomes a
plain sum.

Per-core layout: 8 DMA groups of 16 nodes (2 sub-groups of 8 nodes = 4
node pairs each); each group is one stats batch. Batch (64) x
node-parity packs the 128 SBUF partitions for phase 1, so the rank dim
r stays a free axis and the RMS statistics live 16-per-partition.

Pipelining: group loads are prefetched 4 deep on the SP HWDGE queue so
the DMA engines stream continuously; phase 1 is software-pipelined 2
batches deep, and the output copies/stores are deferred one iteration
so they never head-of-line block the DVE/SP queues. All wire tiles are
permanently SBUF-resident with fixed buffers, and the load schedule
WRAPS across the repeat loop: each For_i iteration reloads groups
0..PRE-1 for the next iteration during its own tail (when the DMA
engines are otherwise idle), so the all-engine For_i barrier no longer
costs a pipeline refill -- worth ~14us/iteration measured. The residual
(sum_c x) is accumulated straight into the phase-2 PSUM tile with
identity-stationary matmuls, so the output path is a single PSUM->SBUF
copy on DVE and the elementwise residual adds disappear.

Per sub: phase-1 matmuls write a per-sub PSUM tile; squares run on ACT
directly from PSUM, the rank pair products on DVE (one operand
evacuated to SBUF -- PSUM has a single DVE read port), and the
r-reduction is a per-sub bf16 tensor_tensor add tree (2x mode). The
m01*m23 merge runs on GPSIMD. The merged tensor is transposed on the
PE with the even sub's r rows on partitions 0:64 and the odd sub's on
64:128 (K-base-64 matmuls are legal for M<=64; only M=128 crashes), so
factor_out loads at full 128-partition DMA width and a single ACT copy
evacuates both subs; the projection runs with fo as the stationary
(M=128 at base 0, two M=64 halves at base 64). The per-node gain is
folded into factor_out on the host (it only scales the projection
term), removing it from the wire and the on-chip scale chain.
"""

import numpy as np
import ml_dtypes

B = 64
N = 1024
C = 4
D = 128
R = 64
NCORES = 8
NS = N // NCORES   # nodes per core (128)
NG = 8             # DMA groups per core (= stats batches)
SG = NS // NG      # nodes per DMA group (16)
NSUB = 2           # sub-groups per DMA group
SUBN = SG // NSUB  # nodes per sub-group (8)
HH = SUBN // 2     # node pairs per sub-group (4)
PRE = 4            # load prefetch depth (groups)
EPS_SCALED = 1e-6 / 16.0
BF16 = ml_dtypes.bfloat16

_CACHE = {}


def _build_nc(repeat=1):
    import concourse.bacc as bacc
    import concourse.tile as tile
    import concourse.mybir as mybir
    from concourse.masks import make_identity

    dt = mybir.dt
    bf = dt.bfloat16
    f32 = dt.float32
    Act = mybir.ActivationFunctionType
    Alu = mybir.AluOpType
    AxX = mybir.AxisListType.X

    nc = bacc.Bacc()
    # x and factors interleaved: [g, i, (xf, sub, c, j, b/r)] ; one DMA per group
    XFW = 2 * NSUB * C * SUBN * 64
    xfp = nc.declare_dram_parameter("xfp", [NG, D, XFW], bf, isOutput=False)
    # factor_out packed: [gpair, (sub, r), (glo/ghi, hh, n2, o)] -- full 128
    # partitions (sub 0's rows at partitions 0:64, sub 1's at 64:128), two
    # groups per DMA so each transfer is 0.5 MB; gain is folded into
    # factor_out on the host, so there is no gain tensor on the wire
    fop = nc.declare_dram_parameter(
        "fop", [NG // 4, 128, 4 * HH * 2 * D], bf, isOutput=False
    )
    # out packed: [gpair, o, (glo/ghi, sub, hh, b2)] -- two groups per DMA
    out = nc.declare_dram_parameter(
        "out_t", [NG // 2, D, 2 * NSUB * HH * 128], bf, isOutput=True
    )

    with tile.TileContext(nc) as tc:
        with tc.tile_pool(name="consts", bufs=1) as consts:
            identity = consts.tile([128, 128], bf)
            make_identity(nc, identity)

            with (
                tc.tile_pool(name="xpool", bufs=1) as xpool,
                tc.tile_pool(name="fopool", bufs=1) as fopool,
                tc.tile_pool(name="opool", bufs=1) as opool,
                tc.tile_pool(name="sqpool", bufs=3) as sqpool,
                tc.tile_pool(name="prodpool", bufs=3) as prodpool,
                tc.tile_pool(name="small", bufs=4) as small,
                tc.tile_pool(name="mgspool", bufs=3) as mgspool,
                tc.tile_pool(name="mtpool", bufs=3) as mtpool,
                tc.tile_pool(name="pps", bufs=2, space="PSUM") as pps,
                tc.tile_pool(name="trps", bufs=2, space="PSUM") as trps,
                tc.tile_pool(name="ops", bufs=2, space="PSUM") as ops,
            ):
                # Every group's wire tiles are permanently resident (fixed
                # buffers, one tag each): the loop body can then reload
                # groups 0..PRE-1 for the NEXT For_i iteration during this
                # iteration's tail, when the DMA engines are otherwise idle.
                # Without this, the all-engine For_i barrier costs a full
                # pipeline refill (~6us) every measured iteration.
                xfts = [
                    xpool.tile(
                        [128, 2, NSUB, C, SUBN * 64], bf, tag=f"xft{g}",
                        name="xft",
                    )
                    for g in range(NG)
                ]
                # fo rides four groups per DMA (1 MB transfers) and the
                # output staging tile spans two groups (one 0.5 MB store per
                # pair) -- small DMAs are descriptor-bound
                fots = [
                    fopool.tile(
                        [128, 4, HH, 2, D], bf, tag=f"fot{q}", name="fot"
                    )
                    for q in range(NG // 4)
                ]
                ots = [
                    opool.tile(
                        [128, 2, NSUB, HH * 128], bf, tag=f"ot{p}", name="ot"
                    )
                    for p in range(NG // 2)
                ]
                gsts = [
                    {
                        "x": xfts[g][:, 0],
                        "f": xfts[g][:, 1],
                        "fo": fots[g // 4][:, g % 4],
                        "ot": ots[g // 2][:, g % 2],
                        "ot2": ots[g // 2],
                    }
                    for g in range(NG)
                ]

                def load(g):
                    nc.sync.dma_start(
                        out=xfts[g].rearrange("p t s c w -> p (t s c w)"),
                        in_=xfp[g],
                    )
                    if g % 4 == 0:
                        nc.sync.dma_start(
                            out=fots[g // 4].rearrange(
                                "p q h n o -> p (q h n o)"
                            ),
                            in_=fop[g // 4],
                        )

                # --- phase 1: matmuls per sub, with per-sub PSUM evacuation
                # so the next sub's matmuls never wait on the ACT queue ---
                def ph1(bst, k):
                    gst = bst["gst"]
                    xt, ft = gst["x"], gst["f"]
                    if k == 0:
                        bst["sq"] = sqpool.tile(
                            [128, 2, HH, C, R], bf, tag="sq", name="sq"
                        )
                        bst["mp"] = prodpool.tile(
                            [128, 2, HH, 2, R], bf, tag="mp", name="mp"
                        )
                    pp = pps.tile([128, HH, C, R], f32, tag="pp", name="pp")
                    for hh in range(HH):
                        for c in range(C):
                            for g2 in range(2):
                                j = 2 * hh + g2
                                nc.tensor.matmul(
                                    pp[64 * g2: 64 * g2 + 64, hh, c, :],
                                    lhsT=xt[:, k, c, 64 * j: 64 * j + 64],
                                    rhs=ft[:, k, c, 64 * j: 64 * j + 64],
                                )
                    # squares (ACT) read PSUM directly; the pair products need
                    # one SBUF operand (PSUM has a single DVE read port), so
                    # only the c=2:4 half is evacuated
                    nc.scalar.activation(
                        out=bst["sq"][:, k], in_=pp, func=Act.Square
                    )
                    pbh = prodpool.tile([128, HH, 2, R], bf, tag="pbh")
                    nc.scalar.copy(out=pbh, in_=pp[:, :, 2:4, :])
                    nc.vector.tensor_mul(bst["mp"][:, k], pp[:, :, 0:2, :], pbh)
                    # per-sub bf16 add tree over r (2x mode): sub 0's tree
                    # overlaps sub 1's matmuls
                    sqk = bst["sq"][:, k].rearrange("p h c r -> p (h c) r")
                    t1 = sqpool.tile([128, HH * C, 32], bf, tag="t1")
                    nc.vector.tensor_add(t1, sqk[:, :, 0:32], sqk[:, :, 32:64])
                    t2 = sqpool.tile([128, HH * C, 16], bf, tag="t2")
                    nc.vector.tensor_add(t2, t1[:, :, 0:16], t1[:, :, 16:32])
                    t3 = bst["t3"] if k else sqpool.tile(
                        [128, 2, HH * C, 8], bf, tag="t3s", name="t3"
                    )
                    bst["t3"] = t3
                    nc.vector.tensor_add(t3[:, k], t2[:, :, 0:8], t2[:, :, 8:16])

                def stats(bst):
                    ssq = small.tile([128, 2, HH, C], f32, tag="ssq")
                    nc.vector.reduce_sum(
                        out=ssq.rearrange("p k h c -> p (k h c)"),
                        in_=bst["t3"].rearrange("p k w e -> p (k w) e"),
                        axis=AxX,
                    )
                    # Pi_c rms_c = sqrt(Pi_c ssq_c) / R^2  (eps is ~1e-6 relative
                    # to msq here -- far below bf16 noise -- so it is dropped)
                    s4 = small.tile([128, 2, HH], f32, tag="s4")
                    nc.vector.tensor_reduce(
                        out=s4.rearrange("p k h -> p (k h)"),
                        in_=ssq.rearrange("p k h c -> p (k h) c"),
                        op=Alu.mult,
                        axis=AxX,
                    )
                    # sqrt(s4)/R^2 then reciprocal, fused via scale = 1/R^4
                    rms4 = small.tile([128, 2, HH], f32, tag="rms4")
                    nc.scalar.activation(
                        out=rms4, in_=s4, func=Act.Sqrt, scale=1.0 / float(R) ** 4
                    )
                    # gain is pre-folded into factor_out on the host, so the
                    # reciprocal IS the merge scale
                    rstd4 = small.tile([128, 2, HH], f32, tag="rstd4")
                    nc.vector.reciprocal(out=rstd4, in_=rms4)
                    bst["scl2"] = rstd4

                def ph2(bst):
                    gst = bst["gst"]
                    # merged = (m01 * m23) * scl2, batched over both subs
                    mga = mgspool.tile([128, 2, HH, R], bf, tag="mga")
                    nc.gpsimd.tensor_mul(
                        mga, bst["mp"][:, :, :, 0, :], bst["mp"][:, :, :, 1, :]
                    )
                    mgs = mgspool.tile([128, 2, HH, R], bf, tag="mgs")
                    scl2b = bst["scl2"].unsqueeze(3).broadcast_to(
                        [128, 2, HH, R]
                    )
                    nc.vector.tensor_mul(mgs, mga, scl2b)
                    # transpose each pair to [r, b2]: sub 0's r rows land on
                    # partitions 0:64, sub 1's on 64:128 (K-base-64 with M<=64
                    # is legal on HW; only M=128 at base 64 crashes), so fo
                    # loads full-width and one ACT copy evacuates both subs
                    mts = mtpool.tile([128, HH, 128], bf, tag="mts")
                    mtp = trps.tile([128, HH, 128], bf, tag="mtp")
                    for k in range(2):
                        for hh in range(HH):
                            nc.tensor.transpose(
                                mtp[64 * k: 64 * k + 64, hh, :],
                                mgs[:, k, hh, :],
                                identity,
                            )
                    nc.scalar.copy(out=mts, in_=mtp)
                    fot = gst["fo"]
                    for k in range(2):
                        op = bst["op"][k]
                        for hh in range(HH):
                            for n2 in range(2):
                                rhs = mts[64 * k: 64 * k + 64, hh,
                                          64 * n2: 64 * n2 + 64]
                                last = hh == HH - 1 and n2 == 1
                                if k == 0:
                                    nc.tensor.matmul(
                                        op[:, hh, 64 * n2: 64 * n2 + 64],
                                        lhsT=fot[0:64, hh, n2, :],
                                        rhs=rhs,
                                        start=False,
                                        stop=last,
                                        skip_group_check=True,
                                    )
                                else:
                                    # K rows at base 64 need M<=64: split o
                                    for oh in range(2):
                                        nc.tensor.matmul(
                                            op[64 * oh: 64 * oh + 64, hh,
                                               64 * n2: 64 * n2 + 64],
                                            lhsT=fot[64:128, hh, n2,
                                                     64 * oh: 64 * oh + 64],
                                            rhs=rhs,
                                            start=False,
                                            stop=(last and oh == 1),
                                            skip_group_check=True,
                                        )

                def resid(bst, k):
                    # residual sum_c x accumulated into PSUM via identity
                    # stationary (x is pre-scaled 1/4 on the host); only needs
                    # the group's x tile, so it keeps the PE queue fed while
                    # the merge chain catches up
                    xt = bst["gst"]["x"]
                    op = ops.tile([128, HH, 128], f32, tag="op", name="op")
                    bst["op"].append(op)
                    opf = op.rearrange("p h w -> p (h w)")
                    for c in range(C):
                        nc.tensor.matmul(
                            opf,
                            lhsT=identity,
                            rhs=xt[:, k, c, :],
                            start=(c == 0),
                            stop=False,
                            skip_group_check=True,
                        )

                def flush(bst, t):
                    # output copies + stores, deferred one iteration so their
                    # projection dependency is long resolved and they never
                    # head-of-line block the DVE/SP queues; per-sub so the
                    # first store overlaps the second copy
                    gst = bst["gst"]
                    for k in range(2):
                        opf = bst["op"][k].rearrange("p h w -> p (h w)")
                        nc.vector.tensor_copy(gst["ot"][:, k], opf)
                    if t % 2 == 1:
                        nc.sync.dma_start(
                            out=out[t // 2],
                            in_=gst["ot2"].rearrange("p q s w -> p (q s w)"),
                        )

                def mkbatch(t):
                    bst = {"gst": gsts[t], "op": [], "mga": []}
                    ph1(bst, 0)
                    ph1(bst, 1)
                    return bst

                def emit_body():
                    # phase 1 pipelined 2 deep; loads run PRE groups ahead
                    # and WRAP: groups 0..PRE-1 are reloaded for the next
                    # For_i iteration during this iteration's tail
                    prev = mkbatch(0)
                    nxt = mkbatch(1)
                    prev2 = None
                    for t in range(NG):
                        if prev2 is not None:
                            flush(prev2, t - 1)
                        resid(prev, 0)
                        resid(prev, 1)
                        stats(prev)
                        ph2(prev)
                        load((t + PRE) % NG)
                        nxt2 = mkbatch(t + 2) if t + 2 < NG else None
                        prev2, prev, nxt = prev, nxt, nxt2
                    flush(prev2, NG - 1)

                # prologue: fill groups 0..PRE-1 once; the loop body keeps
                # them refilled across iterations
                for g in range(PRE):
                    load(g)
                if repeat > 1:
                    with tc.For_i(0, repeat, 1):
                        emit_body()
                else:
                    emit_body()

    nc.compile()
    return nc


def _get_nc(repeat=1):
    key = ("nc", repeat)
    if key not in _CACHE:
        _CACHE[key] = _build_nc(repeat)
    return _CACHE[key]


def _pack_x(x):
    # [B, N, C, D] -> per core [NG, D, (sub, c, j, b)], pre-scaled by 1/4
    a = (np.asarray(x, dtype=np.float32) * 0.25).astype(BF16)
    a = a.reshape(B, NCORES, NG, NSUB, SUBN, C, D)
    a = np.transpose(a, (1, 2, 6, 3, 5, 4, 0))  # [core, g, i, sub, c, j, b]
    return np.ascontiguousarray(a.reshape(NCORES, NG, D, NSUB * C * SUBN * 64))


def _pack_factors(factors):
    # [C, N, R, D] -> per core [NG, D, (sub, c, j, r)]
    f = np.asarray(factors, dtype=np.float32).astype(BF16)
    f = f.reshape(C, NCORES, NG, NSUB, SUBN, R, D)
    f = np.transpose(f, (1, 2, 6, 3, 0, 4, 5))  # [core, g, i, sub, c, j, r]
    return np.ascontiguousarray(f.reshape(NCORES, NG, D, NSUB * C * SUBN * R))


def _pack_factor_out(factor_out, gain):
    # [N, R, D] -> per core [NG//2, (sub, r), (ghalf, hh, n2, o)] on 128
    # partitions, two groups per pair, with the per-node gain folded in
    # (it only scales the projection term)
    q = np.asarray(factor_out, dtype=np.float32) * np.asarray(
        gain, dtype=np.float32
    ).reshape(N, 1, 1)
    q = q.astype(BF16)
    q = q.reshape(NCORES, NG // 4, 4, NSUB, HH, 2, R, D)
    # [core, gquad, sub, r, gq, hh, n2, o]
    q = np.transpose(q, (0, 1, 3, 6, 2, 4, 5, 7))
    return np.ascontiguousarray(
        q.reshape(NCORES, NG // 4, 128, 4 * HH * 2 * D)
    )


def _unpack_out(res_t):
    # [NG//2, D(o), (ghalf, sub, hh, g2, b)] -> [B, NS, D] fp32
    a = np.asarray(res_t).reshape(NG // 2, D, 2, NSUB, HH, 2, 64)
    # [b, gpair, ghalf, sub, hh, g2, o]
    a = np.transpose(a, (6, 0, 2, 3, 4, 5, 1))
    return np.ascontiguousarray(
        a.reshape(64, NS, D).astype(np.float32)
    )


def make_in_maps(x, factors, factor_out, gain):
    x_packed = _pack_x(x)
    f_packed = _pack_factors(factors)
    W = NSUB * C * SUBN * 64
    xf = np.stack([x_packed.reshape(NCORES, NG, D, W),
                   f_packed.reshape(NCORES, NG, D, W)], axis=3)
    xf = np.ascontiguousarray(xf.reshape(NCORES, NG, D, 2 * W))
    fo_packed = _pack_factor_out(factor_out, gain)
    in_maps = []
    for k in range(NCORES):
        in_maps.append(
            {
                "xfp": np.ascontiguousarray(xf[k]),
                "fop": np.ascontiguousarray(fo_packed[k]),
            }
        )
    return in_maps


def kernel(x, factors, factor_out, gain):
    from concourse.bass_utils import run_bass_kernel_spmd

    nc = _get_nc()
    in_maps = make_in_maps(x, factors, factor_out, gain)
    res = run_bass_kernel_spmd(nc, in_maps, core_ids=list(range(NCORES)))
    return np.concatenate(
        [_unpack_out(res.results[k]["out_t"]) for k in range(NCORES)], axis=1
    )
